# revision 1
# baseline (speedup 1.0000x reference)
import ml_dtypes
import numpy as np

import concourse.bass as bass
import concourse.tile as tile
from concourse import bacc, mybir
from concourse.bass_utils import run_bass_kernel_spmd

SL, TL, BS, H = 2048, 2048, 16, 512
NCORES = 8
BPC = BS // NCORES

F32 = mybir.dt.float32
BF16 = mybir.dt.bfloat16
FP8 = mybir.dt.float8e4

NS = SL // 128
NH = H // 128
TCHUNK = 512
NTC = TL // TCHUNK
TPC = TCHUNK // 128
SCALE = 32.0
DSCALE = 16.0


def build():
    nc = bacc.Bacc("TRN2", target_bir_lowering=False, debug=False,
                   num_devices=NCORES)
    out_e = nc.dram_tensor("out_e", [SL, BPC, 2 * H], F32,
                           kind="ExternalInput").ap()
    out_d = nc.dram_tensor("out_d", [TL, BPC, H], F32,
                           kind="ExternalInput").ap()
    ident = nc.dram_tensor("ident", [128, 128], BF16,
                           kind="ExternalInput").ap()
    out = nc.dram_tensor("out", [TL, BPC, H], F32,
                         kind="ExternalOutput").ap()

    exp = mybir.ActivationFunctionType.Exp
    dr = mybir.MatmulPerfMode.DoubleRow

    with tile.TileContext(nc) as tc:
        with (
            tc.tile_pool(name="consts", bufs=1) as consts,
            tc.tile_pool(name="stage_e", bufs=4) as stage_e_pool,
            tc.tile_pool(name="stage_d", bufs=4) as stage_d_pool,
            tc.tile_pool(name="oenat", bufs=2 * NS) as oenat_pool,
            tc.tile_pool(name="oet", bufs=2 * NS) as oet_pool,
            tc.tile_pool(name="odt", bufs=2 * NTC) as odt_pool,
            tc.tile_pool(name="pbuf", bufs=8) as p_pool,
            tc.tile_pool(name="d8buf", bufs=2 * NS) as d8_pool,
            tc.tile_pool(name="oe8buf", bufs=NS) as oe8_pool,
            tc.tile_pool(name="osb", bufs=3) as osb_pool,
            tc.tile_pool(name="small", bufs=4) as small_pool,
            tc.tile_pool(name="psS", bufs=3, space="PSUM") as psS_pool,
            tc.tile_pool(name="psC", bufs=2, space="PSUM") as psC_pool,
            tc.tile_pool(name="psD", bufs=1, space="PSUM") as psD_pool,
            tc.tile_pool(name="ptr", bufs=2, space="PSUM") as ptr_pool,
        ):
            ones = consts.tile([128, 1], BF16, tag="ones")
            nc.vector.memset(ones, 1.0)
            ones8 = consts.tile([128, 2, 1], FP8, tag="ones8")
            nc.vector.memset(ones8, 1.0)
            onesK1 = consts.tile([1, 128], BF16, tag="onesK1")
            nc.vector.memset(onesK1, 1.0)
            denc = consts.tile([1, 1], BF16, tag="denc")
            nc.vector.memset(denc, float(DSCALE * SL))
            idt = consts.tile([128, 128], BF16, tag="idt")
            nc.sync.dma_start(idt, ident)

            warm = consts.tile([128, TCHUNK], BF16, tag="warm")
            nc.vector.memset(warm, 0.25)
            wt = ptr_pool.tile([128, TCHUNK], F32, tag="ptr")
            for _ in range(28):
                nc.tensor.matmul(wt, warm[:, 0:128], warm,
                                 start=True, stop=True)

            def transpose_tiles(src, dst):
                pt = ptr_pool.tile([128, NH * 128], F32, tag="ptr")
                for c in range(NH):
                    nc.tensor.matmul(pt[:, c * 128:(c + 1) * 128],
                                     src[:, c * 128:(c + 1) * 128], idt,
                                     start=True, stop=True)
                nc.vector.tensor_copy(dst, pt)

            class BatchState:
                def __init__(self, b):
                    self.b = b
                    self.oe_tiles = []
                    self.oe8_pairs = []
                    self.oeT_tiles = []
                    self.odT_chunks = []
                    self.d8_pairs = {tci: [] for tci in range(NTC)}
                    self.cs = None

            def load_d(S, ci):
                odc = odt_pool.tile([128, NH, TCHUNK], FP8, tag="odT",
                                    name=f"odT_{S.b}_{ci}")
                S.odT_chunks.append(odc)
                sd = stage_d_pool.tile([128, TPC, H], BF16, tag="sd",
                                       name=f"sd_{S.b}_{ci}")
                src = out_d[ci * TCHUNK:(ci + 1) * TCHUNK, S.b, :]
                nc.gpsimd.dma_start(
                    sd, src.rearrange("(k p) h -> p k h", p=128))
                for k in range(TPC):
                    transpose_tiles(sd[:, k, :],
                                    odc[:, :, k * 128:(k + 1) * 128])

            def load_e(S, j):
                st = stage_e_pool.tile([128, 2, 2 * H], BF16, tag="st",
                                       name=f"st_{S.b}_{j}")
                src = out_e[j * 256:(j + 1) * 256, S.b, :]
                nc.gpsimd.dma_start(
                    st, src.rearrange("(k p) h -> p k h", p=128))
                oe8 = oe8_pool.tile([128, 2, H], FP8, tag="oe8",
                                    name=f"oe8_{S.b}_{j}")
                S.oe8_pairs.append(oe8)
                for k in range(2):
                    oe = oenat_pool.tile([128, H], BF16, tag="oe",
                                         name=f"oe_{S.b}_{2 * j + k}")
                    oeT = oet_pool.tile([128, NH, 128], FP8, tag="oeT",
                                        name=f"oeT_{S.b}_{2 * j + k}")
                    S.oe_tiles.append(oe)
                    S.oeT_tiles.append(oeT)
                    nc.vector.tensor_add(oe, st[:, k, 0:H],
                                         st[:, k, H:2 * H])
                    transpose_tiles(oe, oeT)
                    nc.vector.tensor_copy(oe8[:, k, :], oe)

            def mm1(S, tci, i):
                psS = psS_pool.tile([128, TCHUNK], F32, tag="psS")
                for c2 in range(NH // 2):
                    nc.tensor.matmul(
                        psS,
                        S.oeT_tiles[i][:, 2 * c2:2 * c2 + 2, :],
                        S.odT_chunks[tci][:, 2 * c2:2 * c2 + 2, :],
                        start=(c2 == 0), stop=(c2 == NH // 2 - 1),
                        perf_mode=dr)
                P = p_pool.tile([128, TCHUNK], BF16, tag="P",
                                name=f"P_{S.b}_{tci}_{i}")
                nc.scalar.activation(P, psS, exp,
                                     scale=1.0 / (SCALE * SCALE))
                if i % 2 == 0:
                    d8 = d8_pool.tile([128, 2, TCHUNK], FP8, tag="d8",
                                      name=f"d8_{S.b}_{tci}_{i // 2}")
                    S.d8_pairs[tci].append(d8)
                nc.vector.tensor_scalar(S.d8_pairs[tci][i // 2][:, i % 2, :],
                                        P, -1.0, DSCALE,
                                        mybir.AluOpType.add,
                                        mybir.AluOpType.mult)

            def colsum(S):
                pcs = ptr_pool.tile([1, H], F32, tag="ptr")
                for i in range(NS):
                    nc.tensor.matmul(pcs, ones, S.oe_tiles[i],
                                     start=(i == 0), stop=(i == NS - 1))
                cs = small_pool.tile([1, H], BF16, tag="cs", bufs=2)
                nc.vector.tensor_scalar(cs, pcs, DSCALE, None,
                                        mybir.AluOpType.mult)
                S.cs = cs

            def mm2(S, tci, feed=None):
                for tt in range(TPC):
                    psC = psC_pool.tile([128, H], F32, tag="psC")
                    psD = psD_pool.tile([128, 1], F32, tag="psD")
                    nc.tensor.matmul(psC, onesK1, S.cs,
                                     start=True, stop=False)
                    nc.tensor.matmul(psD, onesK1, denc,
                                     start=True, stop=False)
                    for j in range(NS // 2):
                        if feed is not None:
                            thunk = next(feed, None)
                            if thunk is not None:
                                thunk()
                        lhsT = S.d8_pairs[tci][j][:, :,
                                                  tt * 128:(tt + 1) * 128]
                        nc.tensor.matmul(psC, lhsT, S.oe8_pairs[j],
                                         start=False,
                                         stop=(j == NS // 2 - 1),
                                         perf_mode=dr)
                        nc.tensor.matmul(psD, lhsT, ones8,
                                         start=False,
                                         stop=(j == NS // 2 - 1),
                                         perf_mode=dr)
                    rc = small_pool.tile([128, 1], F32, tag="rc")
                    nc.vector.reciprocal(rc, psD)
                    ob = osb_pool.tile([128, H], F32, tag="ob")
                    nc.vector.tensor_scalar(ob, psC, rc, None,
                                            mybir.AluOpType.mult)
                    t0 = tci * TCHUNK + tt * 128
                    nc.sync.dma_start(out[t0:t0 + 128, S.b, :], ob)

            def head_ops(S, mm1_chunks):
                ops = []
                for ci in range(2):
                    ops.append(lambda S=S, ci=ci: load_d(S, ci))
                for j in range(NS // 2):
                    ops.append(lambda S=S, j=j: load_e(S, j))
                    if 2 + j < NTC:
                        ops.append(lambda S=S, ci=2 + j: load_d(S, ci))
                    if j >= 1:
                        for s in (2 * (j - 1), 2 * j - 1):
                            for tci in range(mm1_chunks):
                                ops.append(
                                    lambda S=S, t=tci, s=s: mm1(S, t, s))
                for s in (NS - 2, NS - 1):
                    for tci in range(mm1_chunks):
                        ops.append(lambda S=S, t=tci, s=s: mm1(S, t, s))
                return ops

            for b in range(BPC):
                S = BatchState(b)
                for op in head_ops(S, NTC):
                    op()
                colsum(S)
                for tci in range(NTC):
                    mm2(S, tci)

    nc.compile()
    return nc


_nc = None
last_result = None
_IDENT = (np.eye(128) * SCALE).astype(ml_dtypes.bfloat16)


def kernel(in_e=None, out_e=None, out_d=None, _trace=False, **_unused):
    global _nc, last_result
    if _nc is None:
        _nc = build()
    out_e = np.asarray(out_e, dtype=np.float32)
    out_d = np.asarray(out_d, dtype=np.float32)
    in_maps = []
    for c in range(NCORES):
        sl = slice(c * BPC, (c + 1) * BPC)
        in_maps.append({
            "out_e": np.ascontiguousarray(out_e[:, sl, :]),
            "out_d": np.ascontiguousarray(out_d[:, sl, :]),
            "ident": _IDENT,
        })
    last_result = run_bass_kernel_spmd(_nc, in_maps,
                                       core_ids=list(range(NCORES)),
                                       trace=_trace)
    return np.concatenate(
        [np.asarray(last_result.results[c]["out"]) for c in range(NCORES)],
        axis=1).astype(np.float32)

